# revision 13
# baseline (speedup 1.0000x reference)
"""Trainium2 Bass kernel for nn_BertEncoder_403726926494.

Reference computation (per batch element):
  - ragged sentence extraction from hidden_states, masked-softmax attention
    pooling per sentence with W_doc            -> doc_pooled [B, D, H]
  - query extraction (rows 1..32), masked-softmax pooling with W_query
    broadcast over D                           -> q_bcast   [B, D, H]

Device strategy (SPMD, one program on 8 cores, 8 batch elements per core):
  - All float staging in bf16 (tolerance 2e-2 >> bf16 error ~1e-3); PSUM
    accumulation and outputs stay f32.
  - TWO examples share each token stream (A rows then B rows) so the
    128-token chunks carry almost no padding: A's sentences use selector
    columns 0..15, B's use 16..31.  4 paired slots x 8 cores = 32 pairs,
    13 doc chunks/core (vs 17 unpaired).  Queries ride 2 extra chunks
    (8 examples x 32 rows) into a separate small PSUM tile.
  - Host packs one dram tensor xall[128, NCH, H+2] per core (770-wide
    chunks keep every chunk 4-byte aligned for DVE 16-bit packing); col
    768 is a ones column for the softmax denominators, col 769 is pad.
  - Scores s[t] = x_t . W: fused DVE scalar_tensor_tensor per chunk
    (out = (x*1)*W, accum_out = s), with per-slot knobs to offload the
    multiply to GpSimd ("gps") or split DVE-multiply/ACT-reduce ("mix").
  - alpha = exp(s + logmask): one ACT EXP per chunk (logmask bf16,
    chunk-contiguous), bias = per-partition score column; output bf16.
  - num[j,:H] | den[j] = alpha^T @ [X | 1]: two PE matmuls (512 + 257
    cols, PSUM bank split) per chunk, bf16; slots 0,1 accumulate in PSUM
    tile A rows 0..63, slots 2,3 in tile B rows 64..127, queries in a
    [32, .] tile.  Each half is normalized (1/(den+eps)) and stored as
    soon as its slots finish, overlapping the rest of the compute.
  - b_doc / b_query shift every score in a softmax segment equally, so
    they cancel and are ignored.
"""

import numpy as np
import ml_dtypes

B, L, H = 64, 512, 768
D, S, Q = 16, 64, 32
NCORES = 8
NSLOTS = 4         # paired slots per core
MPAD = 32
NEG_BIAS = -1.0e30
DEN_EPS = 1.0e-30
BF16 = ml_dtypes.bfloat16

# score engine per slot (4 doc slots + query): "dve" | "mix" | "gps"
SCORE_ENG = ["mix", "dve", "dve", "dve", "dve"]

_compiled: dict = {}


def _slot_geometry(slot_spans):
    nts = [(sp + 127) // 128 for sp in slot_spans]
    coffs = [0]
    for nt in nts:
        coffs.append(coffs[-1] + nt)
    return nts, coffs


def _build(slot_spans):
    """Build + compile the SPMD Bass program for the given per-slot spans."""
    from contextlib import ExitStack

    import concourse.bacc as bacc
    import concourse.tile as tile
    from concourse import mybir

    f32 = mybir.dt.float32
    bf16 = mybir.dt.bfloat16
    MULT = mybir.AluOpType.mult
    ADD = mybir.AluOpType.add
    EXP = mybir.ActivationFunctionType.Exp
    COPY = mybir.ActivationFunctionType.Copy

    nts, coffs = _slot_geometry(slot_spans)
    ntsum = coffs[-1]
    NCH = ntsum + 2
    QC = [ntsum, ntsum + 1]

    nc = bacc.Bacc(
        "TRN2", target_bir_lowering=False, debug=False, num_devices=NCORES
    )
    xall = nc.dram_tensor("xall", [128, NCH, H + 2], bf16, kind="ExternalInput").ap()
    sel = nc.dram_tensor("sel", [128, NCH, MPAD], bf16, kind="ExternalInput").ap()
    wdq = nc.dram_tensor("wdq", [1, 2, H], bf16, kind="ExternalInput").ap()
    out = nc.dram_tensor("out", [160, H], f32, kind="ExternalOutput").ap()

    with tile.TileContext(nc) as tc, ExitStack() as ctx:
        const = ctx.enter_context(tc.tile_pool(name="const", bufs=1))
        nump = ctx.enter_context(tc.tile_pool(name="nump", bufs=2, space="PSUM"))
        qnump = ctx.enter_context(tc.tile_pool(name="qnump", bufs=1, space="PSUM"))

        wrow = const.tile([1, 2, H], bf16)
        nc.scalar.dma_start(out=wrow[:], in_=wdq[:])
        sel_t = const.tile([128, NCH, MPAD], bf16)
        nc.scalar.dma_start(out=sel_t[:], in_=sel[:])
        ones2 = const.tile([1, 128], bf16)
        nc.vector.memset(ones2[:], 1.0)
        wbd_t = const.tile([128, H], bf16)
        wbq_t = const.tile([128, H], bf16)
        wbps = qnump.tile([128, 512], f32, tag="wbps", name="wbps")
        def build_wb(i, wbt_):
            for lo in (0, 512):
                hi = min(lo + 512, H)
                nc.tensor.matmul(
                    wbps[:, 0 : hi - lo], ones2[:], wrow[0:1, i, lo:hi],
                    start=True, stop=True,
                    tile_position=(0, 0), skip_group_check=True,
                )
                nc.scalar.activation(
                    wbt_[:, lo:hi], wbps[:, 0 : hi - lo], COPY,
                    bias=0.0, scale=1.0,
                )

        build_wb(0, wbd_t)
        wb_d = wbd_t[:]
        wb_q = wbq_t[:]

        xt = {}
        for s in range(NSLOTS):
            xt[s] = const.tile([128, nts[s], H + 2], bf16, name=f"x{s}")
        xqt = const.tile([128, 2, H + 2], bf16, name="xq")

        def load_slot(s, eng):
            eng.dma_start(
                out=xt[s][:], in_=xall[:, coffs[s] : coffs[s] + nts[s], :]
            )

        load_slot(1, nc.sync)
        load_slot(0, nc.scalar)
        load_slot(3, nc.sync)
        nc.scalar.dma_start(out=xqt[:], in_=xall[:, ntsum : ntsum + 2, :])
        load_slot(2, nc.sync)

        scol = const.tile([128, NCH], f32)
        at = const.tile([128, NCH, MPAD], bf16)
        scratch = const.tile([128, H], bf16)
        s2 = const.tile([128, H], bf16)
        xwp = ctx.enter_context(tc.tile_pool(name="xwp", bufs=2))
        numA = nump.tile([128, 1024], f32, tag="num", name="numA")
        numB = nump.tile([128, 1024], f32, tag="num", name="numB")
        qnum = qnump.tile([32, 1024], f32, tag="qnum", name="qnum")
        doAB = const.tile([128, H], f32)
        qo = const.tile([32, H], f32)
        de = const.tile([128, 1], f32)
        rec = const.tile([128, 1], f32)
        deq = const.tile([32, 1], f32)
        recq = const.tile([32, 1], f32)

        def emit_chunk_scores(x_ap, wb, cc):
            nc.vector.scalar_tensor_tensor(
                out=scratch[:], in0=x_ap, scalar=1.0, in1=wb,
                op0=MULT, op1=MULT, accum_out=scol[:, cc : cc + 1],
            )

        def emit_multi_scores_eng(x_ap3, wb, cc0, nt, tt_eng):
            # multiply on tt_eng (whole slot), per-chunk accum-reduce on ACT
            xw = xwp.tile([128, nt, H], bf16, tag="xw", name=f"xw{cc0}")
            tt_eng.tensor_tensor(
                out=xw[:], in0=x_ap3,
                in1=wb.rearrange("p (o h) -> p o h", o=1).broadcast_to(
                    [128, nt, H]
                ),
                op=MULT,
            )
            for c in range(nt):
                nc.scalar.activation(
                    s2[:], xw[:, c, :], COPY, bias=0.0, scale=1.0,
                    accum_out=scol[:, cc0 + c : cc0 + c + 1],
                )

        def emit_chunk_alpha(cc):
            nc.scalar.activation(
                at[:, cc, :], sel_t[:, cc, :], EXP,
                bias=scol[:, cc : cc + 1], scale=1.0,
            )

        def emit_chunk_matmuls(numg, w, x_ap, cc, start, stop):
            nc.tensor.matmul(
                numg[32 * w : 32 * w + MPAD, 0:512],
                at[:, cc, :], x_ap[:, 0:512],
                start=start, stop=stop,
                tile_position=(0, 32 * w), skip_group_check=True,
            )
            nc.tensor.matmul(
                numg[32 * w : 32 * w + MPAD, 512 : H + 1],
                at[:, cc, :], x_ap[:, 512 : H + 1],
                start=start, stop=stop,
                tile_position=(0, 32 * w), skip_group_check=True,
            )

        def emit_slot(s):
            nt = nts[s]
            numg = numA if s < 2 else numB
            w = s  # slot0 rows 0..31, slot1 32..63, slot2 64..95, slot3 96..127
            eng = SCORE_ENG[s]
            if eng == "gps":
                emit_multi_scores_eng(
                    xt[s][:, :, 0:H], wb_d, coffs[s], nt, nc.gpsimd
                )
            elif eng == "mix":
                emit_multi_scores_eng(
                    xt[s][:, :, 0:H], wb_d, coffs[s], nt, nc.vector
                )
            for c in range(nt):
                cc = coffs[s] + c
                if eng == "dve":
                    emit_chunk_scores(xt[s][:, c, 0:H], wb_d, cc)
                emit_chunk_alpha(cc)
                emit_chunk_matmuls(
                    numg, w, xt[s][:, c, :], cc, c == 0, c == nt - 1
                )

        def emit_query(b):
            cc = QC[b]
            if SCORE_ENG[4] == "dve":
                emit_chunk_scores(xqt[:, b, 0:H], wb_q, cc)
            else:
                emit_multi_scores_eng(
                    xqt[:, b : b + 1, 0:H], wb_q, cc, 1, nc.vector
                )
            emit_chunk_alpha(cc)
            nc.tensor.matmul(
                qnum[0:32, 0:512], at[:, cc, :], xqt[:, b, 0:512],
                start=(b == 0), stop=(b == 1),
                tile_position=(0, 0), skip_group_check=True,
            )
            nc.tensor.matmul(
                qnum[0:32, 512 : H + 1], at[:, cc, :], xqt[:, b, 512 : H + 1],
                start=(b == 0), stop=(b == 1),
                tile_position=(0, 0), skip_group_check=True,
            )

        def finish_slot(s, eng):
            numg = numA if s < 2 else numB
            lo, hi = 32 * s, 32 * s + 32
            nc.vector.tensor_scalar(
                out=de[lo:hi], in0=numg[lo:hi, H : H + 1],
                scalar1=DEN_EPS, scalar2=None, op0=ADD,
            )
            nc.vector.reciprocal(rec[lo:hi], de[lo:hi])
            nc.scalar.activation(
                doAB[lo:hi, :], numg[lo:hi, 0:H], COPY,
                bias=0.0, scale=rec[lo:hi, 0:1],
            )
            eng.dma_start(out=out[lo:hi, :], in_=doAB[lo:hi, :])

        def finish_query(eng):
            nc.vector.tensor_scalar(
                out=deq[:], in0=qnum[:, H : H + 1],
                scalar1=DEN_EPS, scalar2=None, op0=ADD,
            )
            nc.vector.reciprocal(recq[:], deq[:])
            nc.scalar.activation(
                qo[:], qnum[:, 0:H], COPY, bias=0.0, scale=recq[:, 0:1]
            )
            eng.dma_start(out=out[128:160, :], in_=qo[:])

        emit_slot(1)
        build_wb(1, wbq_t)
        emit_slot(0)
        finish_slot(1, nc.sync)
        finish_slot(0, nc.sync)
        emit_slot(3)
        finish_slot(3, nc.sync)
        emit_query(0)
        emit_query(1)
        finish_query(nc.sync)
        emit_slot(2)
        finish_slot(2, nc.sync)

    nc.compile()
    return nc


def _prepare(query_len, seq_lens):
    """Host-side geometry: spans, pairing, slot assignment, selector masks."""
    ql = np.asarray(query_len).astype(np.int64)
    sl = np.asarray(seq_lens).astype(np.int64)
    offs = ql[:, None] + 2 + np.cumsum(sl, axis=1) - sl  # [B, D] sentence starts
    end = ql + 2 + sl.sum(axis=1)
    span = np.maximum(end, 1 + Q).astype(np.int64)
    order = np.argsort(-span, kind="stable")
    # balanced pairing: rank i with rank 63-i
    pairs = [(int(order[i]), int(order[63 - i])) for i in range(32)]
    pairspan = np.array([span[a] + span[b] for a, b in pairs])
    porder = np.argsort(-pairspan, kind="stable")
    slot_spans = tuple(int(pairspan[porder[8 * s]]) for s in range(NSLOTS))
    nts, coffs = _slot_geometry(slot_spans)
    ntsum = coffs[-1]

    # ex_map[c, s] = (eA, eB) for pair rank 8s+c
    ex_map = np.empty((NCORES, NSLOTS, 2), np.int64)
    sel_all = np.full((NCORES, 128, ntsum + 2, MPAD), NEG_BIAS, np.float32)
    for c in range(NCORES):
        for s in range(NSLOTS):
            eA, eB = pairs[int(porder[8 * s + c])]
            ex_map[c, s] = (eA, eB)
            for h, e in enumerate((eA, eB)):
                base = 0 if h == 0 else int(span[eA])
                for j in range(D):
                    ln = int(sl[e, j])
                    if ln == 0:
                        continue
                    t = base + int(offs[e, j]) + np.arange(ln)
                    sel_all[c, t % 128, coffs[s] + t // 128, 16 * h + j] = 0.0
                # query chunk: example index e2 = 2s+h -> chunk e2//4, block e2%4
                e2 = 2 * s + h
                b, k = divmod(e2, 4)
                sel_all[
                    c, 32 * k : 32 * k + int(ql[e]), ntsum + b, e2
                ] = 0.0
    return slot_spans, ex_map, sel_all, span


def kernel(hidden_states, W_doc, b_doc, W_query, b_query, query_len, seq_lens):
    hs = np.asarray(hidden_states, dtype=np.float32)
    wdq_host = np.stack(
        [
            np.asarray(W_doc, np.float32).reshape(H),
            np.asarray(W_query, np.float32).reshape(H),
        ]
    )[None].astype(BF16)

    slot_spans, ex_map, sel_all, span = _prepare(query_len, seq_lens)

    nc = _compiled.get(slot_spans)
    if nc is None:
        nc = _build(slot_spans)
        _compiled[slot_spans] = nc

    nts, coffs = _slot_geometry(slot_spans)
    ntsum = coffs[-1]
    NCH = ntsum + 2

    in_maps = []
    for c in range(NCORES):
        xbuf = np.zeros((128, NCH, H + 2), np.float32)
        xbuf[:, :, H] = 1.0
        for s in range(NSLOTS):
            eA, eB = int(ex_map[c, s, 0]), int(ex_map[c, s, 1])
            spA, spB = int(span[eA]), int(span[eB])
            nt = nts[s]
            rows = np.zeros((nt * 128, H), np.float32)
            rows[:spA] = hs[eA, :spA]
            rows[spA : spA + spB] = hs[eB, :spB]
            xbuf[:, coffs[s] : coffs[s] + nt, 0:H] = (
                rows.reshape(nt, 128, H).transpose(1, 0, 2)
            )
            for h, e in enumerate((eA, eB)):
                e2 = 2 * s + h
                b, k = divmod(e2, 4)
                xbuf[32 * k : 32 * k + 32, ntsum + b, 0:H] = hs[e, 1 : 1 + Q]
        in_maps.append(
            {
                "xall": xbuf.astype(BF16),
                "sel": sel_all[c].astype(BF16),
                "wdq": wdq_host,
            }
        )

    from concourse.bass_utils import run_bass_kernel_spmd

    res = run_bass_kernel_spmd(nc, in_maps, list(range(NCORES)))

    doc = np.empty((B, D, H), np.float32)
    qp = np.empty((B, H), np.float32)
    for c in range(NCORES):
        r = res.results[c]
        for s in range(NSLOTS):
            for h in range(2):
                e = int(ex_map[c, s, h])
                doc[e] = r["out"][32 * s + 16 * h : 32 * s + 16 * h + D, :]
                qp[e] = r["out"][128 + 2 * s + h, :]
    q_bcast = np.broadcast_to(qp[:, None, :], (B, D, H))
    return doc, q_bcast


# revision 14
# speedup vs baseline: 1.0954x; 1.0954x over previous
"""Trainium2 Bass kernel for nn_BertEncoder_403726926494.

Reference computation (per batch element):
  - ragged sentence extraction from hidden_states, masked-softmax attention
    pooling per sentence with W_doc            -> doc_pooled [B, D, H]
  - query extraction (rows 1..32), masked-softmax pooling with W_query
    broadcast over D                           -> q_bcast   [B, D, H]

Device strategy (SPMD, one program on 8 cores, 8 batch elements per core):
  - All float staging in bf16 (tolerance 2e-2 >> bf16 error ~1e-3); PSUM
    accumulation and outputs stay f32.
  - TWO examples share each token stream (A rows then B rows) so the
    128-token chunks carry almost no padding: A's sentences use selector
    columns 0..15, B's use 16..31.  4 paired slots x 8 cores = 32 pairs,
    13 doc chunks/core (vs 17 unpaired).  Queries ride 2 extra chunks
    (8 examples x 32 rows) into a separate small PSUM tile.
  - Host packs one dram tensor xall[128, NCH, H+2] per core (770-wide
    chunks keep every chunk 4-byte aligned for DVE 16-bit packing); col
    768 is a ones column for the softmax denominators, col 769 is pad.
  - Scores s[t] = x_t . W: fused DVE scalar_tensor_tensor per chunk
    (out = (x*1)*W, accum_out = s), with per-slot knobs to offload the
    multiply to GpSimd ("gps") or split DVE-multiply/ACT-reduce ("mix").
  - alpha = exp(s + logmask): one ACT EXP per chunk (logmask bf16,
    chunk-contiguous), bias = per-partition score column; output bf16.
  - num[j,:H] | den[j] = alpha^T @ [X | 1]: two PE matmuls (512 + 257
    cols, PSUM bank split) per chunk, bf16; slots 0,1 accumulate in PSUM
    tile A rows 0..63, slots 2,3 in tile B rows 64..127, queries in a
    [32, .] tile.  Each half is normalized (1/(den+eps)) and stored as
    soon as its slots finish, overlapping the rest of the compute.
  - b_doc / b_query shift every score in a softmax segment equally, so
    they cancel and are ignored.
"""

import numpy as np
import ml_dtypes

B, L, H = 64, 512, 768
D, S, Q = 16, 64, 32
NCORES = 8
NSLOTS = 4         # paired slots per core
MPAD = 32
NEG_BIAS = -1.0e30
DEN_EPS = 1.0e-30
BF16 = ml_dtypes.bfloat16

# score engine per slot (4 doc slots + query): "dve" | "mix" | "gps"
SCORE_ENG = ["mix", "dve", "dve", "dve", "dve"]

_compiled: dict = {}


def _slot_geometry(slot_spans):
    nts = [(sp + 127) // 128 for sp in slot_spans]
    coffs = [0]
    for nt in nts:
        coffs.append(coffs[-1] + nt)
    return nts, coffs


def _build(slot_spans):
    """Build + compile the SPMD Bass program for the given per-slot spans."""
    from contextlib import ExitStack

    import concourse.bacc as bacc
    import concourse.tile as tile
    from concourse import mybir

    f32 = mybir.dt.float32
    bf16 = mybir.dt.bfloat16
    MULT = mybir.AluOpType.mult
    ADD = mybir.AluOpType.add
    EXP = mybir.ActivationFunctionType.Exp
    COPY = mybir.ActivationFunctionType.Copy

    nts, coffs = _slot_geometry(slot_spans)
    ntsum = coffs[-1]
    NCH = ntsum + 2
    QC = [ntsum, ntsum + 1]

    nc = bacc.Bacc(
        "TRN2", target_bir_lowering=False, debug=False, num_devices=NCORES
    )
    xall = nc.dram_tensor("xall", [128, NCH, H + 2], bf16, kind="ExternalInput").ap()
    sel = nc.dram_tensor("sel", [128, NCH, MPAD], bf16, kind="ExternalInput").ap()
    wdq = nc.dram_tensor("wdq", [1, 2, H], bf16, kind="ExternalInput").ap()
    out = nc.dram_tensor("out", [160, H], f32, kind="ExternalOutput").ap()

    with tile.TileContext(nc) as tc, ExitStack() as ctx:
        const = ctx.enter_context(tc.tile_pool(name="const", bufs=1))
        nump = ctx.enter_context(tc.tile_pool(name="nump", bufs=2, space="PSUM"))
        qnump = ctx.enter_context(tc.tile_pool(name="qnump", bufs=1, space="PSUM"))

        wrow = const.tile([1, 2, H], bf16)
        nc.scalar.dma_start(out=wrow[:], in_=wdq[:])
        sel_t = const.tile([128, NCH, MPAD], bf16)
        nc.sync.dma_start(out=sel_t[:], in_=sel[:])
        ones2 = const.tile([1, 128], bf16)
        nc.vector.memset(ones2[:], 1.0)
        wbd_t = const.tile([128, H], bf16)
        wbq_t = const.tile([128, H], bf16)
        wbps = qnump.tile([128, 512], f32, tag="wbps", name="wbps")
        def build_wb(i, wbt_):
            for lo in (0, 512):
                hi = min(lo + 512, H)
                nc.tensor.matmul(
                    wbps[:, 0 : hi - lo], ones2[:], wrow[0:1, i, lo:hi],
                    start=True, stop=True,
                    tile_position=(0, 0), skip_group_check=True,
                )
                nc.scalar.activation(
                    wbt_[:, lo:hi], wbps[:, 0 : hi - lo], COPY,
                    bias=0.0, scale=1.0,
                )

        build_wb(0, wbd_t)
        wb_d = wbd_t[:]
        wb_q = wbq_t[:]

        xt = {}
        for s in range(NSLOTS):
            xt[s] = const.tile([128, nts[s], H + 2], bf16, name=f"x{s}")
        xqt = const.tile([128, 2, H + 2], bf16, name="xq")

        def load_slot(s, eng):
            eng.dma_start(
                out=xt[s][:], in_=xall[:, coffs[s] : coffs[s] + nts[s], :]
            )

        # slot1 computes first: land its chunk 0 ahead of the rest
        nc.sync.dma_start(
            out=xt[1][:, 0:1, :], in_=xall[:, coffs[1] : coffs[1] + 1, :]
        )
        nc.sync.dma_start(
            out=xt[1][:, 1 : nts[1], :],
            in_=xall[:, coffs[1] + 1 : coffs[1] + nts[1], :],
        )
        load_slot(0, nc.scalar)
        load_slot(3, nc.sync)
        nc.sync.dma_start(out=xqt[:], in_=xall[:, ntsum : ntsum + 2, :])
        load_slot(2, nc.sync)

        scol = const.tile([128, NCH], f32)
        at = const.tile([128, NCH, MPAD], bf16)
        scratch = const.tile([128, H], bf16)
        s2 = const.tile([128, H], bf16)
        xwp = ctx.enter_context(tc.tile_pool(name="xwp", bufs=2))
        numA = nump.tile([128, 1024], f32, tag="num", name="numA")
        numB = nump.tile([128, 1024], f32, tag="num", name="numB")
        qnum = qnump.tile([32, 1024], f32, tag="qnum", name="qnum")
        doAB = const.tile([128, H], f32)
        qo = const.tile([32, H], f32)
        de = const.tile([128, 1], f32)
        rec = const.tile([128, 1], f32)
        deq = const.tile([32, 1], f32)
        recq = const.tile([32, 1], f32)

        def emit_chunk_scores(x_ap, wb, cc):
            nc.vector.scalar_tensor_tensor(
                out=scratch[:], in0=x_ap, scalar=1.0, in1=wb,
                op0=MULT, op1=MULT, accum_out=scol[:, cc : cc + 1],
            )

        def emit_multi_scores_eng(x_ap3, wb, cc0, nt, tt_eng):
            # multiply on tt_eng (whole slot), per-chunk accum-reduce on ACT
            xw = xwp.tile([128, nt, H], bf16, tag="xw", name=f"xw{cc0}")
            tt_eng.tensor_tensor(
                out=xw[:], in0=x_ap3,
                in1=wb.rearrange("p (o h) -> p o h", o=1).broadcast_to(
                    [128, nt, H]
                ),
                op=MULT,
            )
            for c in range(nt):
                nc.scalar.activation(
                    s2[:], xw[:, c, :], COPY, bias=0.0, scale=1.0,
                    accum_out=scol[:, cc0 + c : cc0 + c + 1],
                )

        def emit_chunk_alpha(cc):
            nc.scalar.activation(
                at[:, cc, :], sel_t[:, cc, :], EXP,
                bias=scol[:, cc : cc + 1], scale=1.0,
            )

        def emit_chunk_matmuls(numg, w, x_ap, cc, start, stop):
            nc.tensor.matmul(
                numg[32 * w : 32 * w + MPAD, 0:512],
                at[:, cc, :], x_ap[:, 0:512],
                start=start, stop=stop,
                tile_position=(0, 32 * w), skip_group_check=True,
            )
            nc.tensor.matmul(
                numg[32 * w : 32 * w + MPAD, 512 : H + 1],
                at[:, cc, :], x_ap[:, 512 : H + 1],
                start=start, stop=stop,
                tile_position=(0, 32 * w), skip_group_check=True,
            )

        def emit_slot(s):
            nt = nts[s]
            numg = numA if s < 2 else numB
            w = s  # slot0 rows 0..31, slot1 32..63, slot2 64..95, slot3 96..127
            eng = SCORE_ENG[s]
            if eng == "gps":
                emit_multi_scores_eng(
                    xt[s][:, :, 0:H], wb_d, coffs[s], nt, nc.gpsimd
                )
            elif eng == "mix":
                emit_multi_scores_eng(
                    xt[s][:, :, 0:H], wb_d, coffs[s], nt, nc.vector
                )
            for c in range(nt):
                cc = coffs[s] + c
                if eng == "dve":
                    emit_chunk_scores(xt[s][:, c, 0:H], wb_d, cc)
                emit_chunk_alpha(cc)
                emit_chunk_matmuls(
                    numg, w, xt[s][:, c, :], cc, c == 0, c == nt - 1
                )

        def emit_query(b):
            cc = QC[b]
            if SCORE_ENG[4] == "dve":
                emit_chunk_scores(xqt[:, b, 0:H], wb_q, cc)
            else:
                emit_multi_scores_eng(
                    xqt[:, b : b + 1, 0:H], wb_q, cc, 1, nc.vector
                )
            emit_chunk_alpha(cc)
            nc.tensor.matmul(
                qnum[0:32, 0:512], at[:, cc, :], xqt[:, b, 0:512],
                start=(b == 0), stop=(b == 1),
                tile_position=(0, 0), skip_group_check=True,
            )
            nc.tensor.matmul(
                qnum[0:32, 512 : H + 1], at[:, cc, :], xqt[:, b, 512 : H + 1],
                start=(b == 0), stop=(b == 1),
                tile_position=(0, 0), skip_group_check=True,
            )

        def finish_slot(s, eng):
            numg = numA if s < 2 else numB
            lo, hi = 32 * s, 32 * s + 32
            nc.vector.tensor_scalar(
                out=de[lo:hi], in0=numg[lo:hi, H : H + 1],
                scalar1=DEN_EPS, scalar2=None, op0=ADD,
            )
            nc.vector.reciprocal(rec[lo:hi], de[lo:hi])
            nc.scalar.activation(
                doAB[lo:hi, :], numg[lo:hi, 0:H], COPY,
                bias=0.0, scale=rec[lo:hi, 0:1],
            )
            eng.dma_start(out=out[lo:hi, :], in_=doAB[lo:hi, :])

        def finish_query(eng):
            nc.vector.tensor_scalar(
                out=deq[:], in0=qnum[:, H : H + 1],
                scalar1=DEN_EPS, scalar2=None, op0=ADD,
            )
            nc.vector.reciprocal(recq[:], deq[:])
            nc.scalar.activation(
                qo[:], qnum[:, 0:H], COPY, bias=0.0, scale=recq[:, 0:1]
            )
            eng.dma_start(out=out[128:160, :], in_=qo[:])

        emit_slot(1)
        build_wb(1, wbq_t)
        emit_slot(0)
        finish_slot(1, nc.sync)
        finish_slot(0, nc.sync)
        emit_slot(3)
        finish_slot(3, nc.sync)
        emit_query(0)
        emit_query(1)
        finish_query(nc.sync)
        emit_slot(2)
        finish_slot(2, nc.sync)

    nc.compile()
    return nc


def _prepare(query_len, seq_lens):
    """Host-side geometry: spans, pairing, slot assignment, selector masks."""
    ql = np.asarray(query_len).astype(np.int64)
    sl = np.asarray(seq_lens).astype(np.int64)
    offs = ql[:, None] + 2 + np.cumsum(sl, axis=1) - sl  # [B, D] sentence starts
    end = ql + 2 + sl.sum(axis=1)
    span = np.maximum(end, 1 + Q).astype(np.int64)
    order = np.argsort(-span, kind="stable")
    # balanced pairing: rank i with rank 63-i
    pairs = [(int(order[i]), int(order[63 - i])) for i in range(32)]
    pairspan = np.array([span[a] + span[b] for a, b in pairs])
    porder = np.argsort(-pairspan, kind="stable")
    slot_spans = tuple(int(pairspan[porder[8 * s]]) for s in range(NSLOTS))
    nts, coffs = _slot_geometry(slot_spans)
    ntsum = coffs[-1]

    # ex_map[c, s] = (eA, eB) for pair rank 8s+c
    ex_map = np.empty((NCORES, NSLOTS, 2), np.int64)
    sel_all = np.full((NCORES, 128, ntsum + 2, MPAD), NEG_BIAS, np.float32)
    for c in range(NCORES):
        for s in range(NSLOTS):
            eA, eB = pairs[int(porder[8 * s + c])]
            ex_map[c, s] = (eA, eB)
            for h, e in enumerate((eA, eB)):
                base = 0 if h == 0 else int(span[eA])
                for j in range(D):
                    ln = int(sl[e, j])
                    if ln == 0:
                        continue
                    t = base + int(offs[e, j]) + np.arange(ln)
                    sel_all[c, t % 128, coffs[s] + t // 128, 16 * h + j] = 0.0
                # query chunk: example index e2 = 2s+h -> chunk e2//4, block e2%4
                e2 = 2 * s + h
                b, k = divmod(e2, 4)
                sel_all[
                    c, 32 * k : 32 * k + int(ql[e]), ntsum + b, e2
                ] = 0.0
    return slot_spans, ex_map, sel_all, span


def kernel(hidden_states, W_doc, b_doc, W_query, b_query, query_len, seq_lens):
    hs = np.asarray(hidden_states, dtype=np.float32)
    wdq_host = np.stack(
        [
            np.asarray(W_doc, np.float32).reshape(H),
            np.asarray(W_query, np.float32).reshape(H),
        ]
    )[None].astype(BF16)

    slot_spans, ex_map, sel_all, span = _prepare(query_len, seq_lens)

    nc = _compiled.get(slot_spans)
    if nc is None:
        nc = _build(slot_spans)
        _compiled[slot_spans] = nc

    nts, coffs = _slot_geometry(slot_spans)
    ntsum = coffs[-1]
    NCH = ntsum + 2

    in_maps = []
    for c in range(NCORES):
        xbuf = np.zeros((128, NCH, H + 2), np.float32)
        xbuf[:, :, H] = 1.0
        for s in range(NSLOTS):
            eA, eB = int(ex_map[c, s, 0]), int(ex_map[c, s, 1])
            spA, spB = int(span[eA]), int(span[eB])
            nt = nts[s]
            rows = np.zeros((nt * 128, H), np.float32)
            rows[:spA] = hs[eA, :spA]
            rows[spA : spA + spB] = hs[eB, :spB]
            xbuf[:, coffs[s] : coffs[s] + nt, 0:H] = (
                rows.reshape(nt, 128, H).transpose(1, 0, 2)
            )
            for h, e in enumerate((eA, eB)):
                e2 = 2 * s + h
                b, k = divmod(e2, 4)
                xbuf[32 * k : 32 * k + 32, ntsum + b, 0:H] = hs[e, 1 : 1 + Q]
        in_maps.append(
            {
                "xall": xbuf.astype(BF16),
                "sel": sel_all[c].astype(BF16),
                "wdq": wdq_host,
            }
        )

    from concourse.bass_utils import run_bass_kernel_spmd

    res = run_bass_kernel_spmd(nc, in_maps, list(range(NCORES)))

    doc = np.empty((B, D, H), np.float32)
    qp = np.empty((B, H), np.float32)
    for c in range(NCORES):
        r = res.results[c]
        for s in range(NSLOTS):
            for h in range(2):
                e = int(ex_map[c, s, h])
                doc[e] = r["out"][32 * s + 16 * h : 32 * s + 16 * h + D, :]
                qp[e] = r["out"][128 + 2 * s + h, :]
    q_bcast = np.broadcast_to(qp[:, None, :], (B, D, H))
    return doc, q_bcast
